# revision 17
# baseline (speedup 1.0000x reference)
"""Trainium2 Bass kernel for nn_CriticAttention (8-core data-parallel), v2.

Math (per reference.py):
  cur  = state[:, ai, :]                       # [B, D]
  s_enc = leaky(bn(cur, axes=0) @ Ws + bs)     # [B, Hid]
  others = state minus agent ai                # [B, A-1, D]
  sa_enc = leaky(bn(others, axes=(0,1)) @ Wc + bc)
  k = einsum('ban,hnd->bhad', sa_enc, Wk)
  v = leaky(einsum('ban,hnd->bhad', sa_enc, Wv))
  q = einsum('bn,hnd->bhd', s_enc, Wq)
  att = softmax(q.k/sqrt(hd)) @ v  -> [B, H*hd]

v2 design (vs v1 baseline at 619us):
  - Phase A: f32 state chunks DMA straight to SBUF, transposed on the (idle)
    PE via identity-matmul transposes, drained f32->bf16 by ScalarE into xT
    tiles; bn_stats on DVE per chunk; xT chunks round-trip through a DRAM
    scratch so phase B re-reads them with cheap contiguous DMAs.  Cross-core
    stats use a single small AllReduce(add) instead of AllGather+sum.
  - Phase B: K and Q projections run in fp8e4 DoubleRow mode (2x PE rate;
    error only perturbs softmax scores, negligible).  V/encoder stay bf16.
    The K PSUM drain is fused into the attention q*k multiply (DVE reads
    PSUM directly).  e*v multiply + reduce run on the Pool engine; encoder /
    V / Q drains on ScalarE.  PSUM: enc 2x[128,1024] + kv8 2x[128,512] +
    v 2x[128,512] = 8 banks.
"""

import os
import sys

import numpy as np

if "/opt/trn_rl_repo" not in sys.path:
    sys.path.insert(0, "/opt/trn_rl_repo")

NCORES = 8
B, A, D, Hid, H, HD = 8192, 16, 256, 512, 8, 64
BL = B // NCORES          # batch per core
CB = 128                  # batch per chunk
NCH = BL // CB            # chunks per core
ACB = A * CB              # 2048 (a-major, b) columns per chunk
NT = Hid // 128           # Hid partition tiles
DT = D // 128             # D partition tiles
AO = A - 1                # number of "other" agents
EPS = 1e-3
ALPHA = 0.3

_CACHE = {}
DEBUG_TAPS = False


def _pieces(ai):
    """Column pieces over the 2048-wide (a-major, b) chunk, split so no piece
    crosses a 512 (PSUM-bank) boundary.  Returns list of (c0, cn, kind) with
    kind 'o' (others, Wc path) or 's' (current agent, Ws path)."""
    out = []
    for lo, hi in ((0, ai * CB), ((ai + 1) * CB, A * CB)):
        c = lo
        while c < hi:
            n = min(hi - c, 512 - (c % 512))
            out.append((c, n, "o"))
            c += n
    out.append((ai * CB, CB, "s"))
    out.sort()
    return out


def _build(ai: int):
    if ai in _CACHE:
        return _CACHE[ai]

    import concourse.bass as bass
    import concourse.tile as tile
    from concourse import bacc, mybir

    f32 = mybir.dt.float32
    bf16 = mybir.dt.bfloat16
    f8 = mybir.dt.float8e4
    Alu = mybir.AluOpType
    Act = mybir.ActivationFunctionType
    DR = mybir.MatmulPerfMode.DoubleRow

    nc = bacc.Bacc("TRN2", target_bir_lowering=False, debug=False,
                   num_devices=NCORES, name="critic_attention")

    state = nc.dram_tensor("state", [BL, A, D], f32, kind="ExternalInput")
    Ws_d = nc.dram_tensor("Ws", [D, Hid], f32, kind="ExternalInput")
    bs_d = nc.dram_tensor("bs", [Hid], f32, kind="ExternalInput")
    Wc_d = nc.dram_tensor("Wc", [D, Hid], f32, kind="ExternalInput")
    bc_d = nc.dram_tensor("bc", [Hid], f32, kind="ExternalInput")
    Wk_d = nc.dram_tensor("Wk", [H, Hid, HD], f32, kind="ExternalInput")
    Wq_d = nc.dram_tensor("Wq", [H, Hid, HD], f32, kind="ExternalInput")
    Wv_d = nc.dram_tensor("Wv", [H, Hid, HD], f32, kind="ExternalInput")
    ident_d = nc.dram_tensor("ident", [128, 128], f32, kind="ExternalInput")
    out_d = nc.dram_tensor("out", [BL, H * HD], f32, kind="ExternalOutput")
    if DEBUG_TAPS:
        dbg_xT = nc.dram_tensor("dbg_xT", [128, DT, ACB], f32, kind="ExternalOutput")
        dbg_s4 = nc.dram_tensor("dbg_s4", [128, DT, 2], f32, kind="ExternalOutput")
        dbg_nms = nc.dram_tensor("dbg_nms", [128, DT, 2], f32, kind="ExternalOutput")
        dbg_sa = nc.dram_tensor("dbg_sa", [128, NT, AO * CB], f32, kind="ExternalOutput")
        dbg_q = nc.dram_tensor("dbg_q", [128, H * HD], f32, kind="ExternalOutput")
        dbg_sc = nc.dram_tensor("dbg_sc", [128, H, AO], f32, kind="ExternalOutput")

    pieces = _pieces(ai)
    oth = [(c0, cn) for c0, cn, k in pieces if k == "o"]
    noth = len(oth)

    def compact(c0):
        # column offset in the ai-compacted (AO, CB) layout
        return c0 if c0 < ai * CB else c0 - CB

    with tile.TileContext(nc) as tc:
        with (
            tc.tile_pool(name="consts", bufs=1) as consts,
            tc.tile_pool(name="dram", bufs=1, space="DRAM") as dram,
            tc.tile_pool(name="xf_pool", bufs=2) as xf_pool,
            tc.tile_pool(name="xs_pool", bufs=2) as xs_pool,
            tc.tile_pool(name="xb_pool", bufs=3) as xb_pool,
            tc.tile_pool(name="sa_pool", bufs=2) as sa_pool,
            tc.tile_pool(name="at_pool", bufs=2) as at_pool,
            tc.tile_pool(name="psum", bufs=2, space="PSUM") as psum,
        ):
            # ---------------- constants / weights ----------------
            ident = consts.tile([128, 128], f32)
            nc.scalar.dma_start(ident[:, :], ident_d[:, :])
            Wc32 = consts.tile([128, DT, Hid], f32)
            Ws32 = consts.tile([128, DT, Hid], f32)
            for dt in range(DT):
                nc.scalar.dma_start(Wc32[:, dt, :], Wc_d[dt * 128:(dt + 1) * 128, :])
                nc.scalar.dma_start(Ws32[:, dt, :], Ws_d[dt * 128:(dt + 1) * 128, :])

            Wk2 = consts.tile([128, NT, H * HD], bf16)
            Wq2 = consts.tile([128, NT, H * HD], bf16)
            Wv2 = consts.tile([128, NT, H * HD], bf16)
            for w_d, w_sb in ((Wk_d, Wk2), (Wq_d, Wq2), (Wv_d, Wv2)):
                for kt in range(NT):
                    src = w_d[:, kt * 128:(kt + 1) * 128, :].rearrange("h p d -> p h d")
                    nc.gpsimd.dma_start(w_sb[:, kt, :].rearrange("p (h d) -> p h d", h=H), src)
            Wk8 = consts.tile([128, NT, H * HD], f8)
            Wq8 = consts.tile([128, NT, H * HD], f8)
            nc.gpsimd.tensor_copy(Wk8[:, :, :], Wk2[:, :, :])
            nc.gpsimd.tensor_copy(Wq8[:, :, :], Wq2[:, :, :])

            bcT = consts.tile([128, NT], f32)
            bsT = consts.tile([128, NT], f32)
            with nc.allow_non_contiguous_dma("tiny bias transpose loads"):
                nc.gpsimd.dma_start(bcT[:, :], bc_d.rearrange("(j p) -> p j", p=128))
                nc.gpsimd.dma_start(bsT[:, :], bs_d.rearrange("(j p) -> p j", p=128))

            # bf16 transposed-x scratch in DRAM: [chunk][dt][d-part][a,b]
            xbfT = dram.tile([NCH, DT, 128, ACB], bf16)

            # ---------------- phase A: load, PE-transpose, stats ----------------
            stato = consts.tile([128, DT, NCH, noth, 6], f32)
            statc = consts.tile([128, DT, NCH, 6], f32)
            for t in range(NCH):
                sview = state[t * CB:(t + 1) * CB, :, :].rearrange("b a d -> b (a d)")
                xf = []
                for hh in range(2):
                    xt = xf_pool.tile([128, 8 * D], f32, tag="xf",
                                      name=f"xf_{t}_{hh}")
                    nc.sync.dma_start(xt[:, :], sview[:, hh * 8 * D:(hh + 1) * 8 * D])
                    xf.append(xt)

                xT = xs_pool.tile([128, DT, ACB], bf16, tag="xT", name=f"xT_{t}")
                for dt in range(DT):
                    for hh in range(2):
                        pe = psum.tile([128, 1024], f32, tag="enc",
                                       name=f"ptr_{t}_{dt}_{hh}")
                        for i in range(8):
                            a = hh * 8 + i
                            nc.tensor.transpose(
                                pe[:, i * 128:(i + 1) * 128],
                                xf[hh][:, (a - hh * 8) * D + dt * 128:
                                       (a - hh * 8) * D + dt * 128 + 128],
                                ident[:, :])
                        for w in range(2):
                            nc.scalar.copy(
                                xT[:, dt, hh * 1024 + w * 512: hh * 1024 + (w + 1) * 512],
                                pe[:, w * 512:(w + 1) * 512])
                    for i, (c0, cn) in enumerate(oth):
                        nc.vector.bn_stats(stato[:, dt, t, i, :],
                                           xT[:, dt, c0:c0 + cn])
                    nc.vector.bn_stats(statc[:, dt, t, :],
                                       xT[:, dt, ai * CB:(ai + 1) * CB])
                    nc.gpsimd.dma_start(xbfT[t, dt], xT[:, dt, :])
                    if DEBUG_TAPS and t == 0:
                        nc.gpsimd.dma_start(dbg_xT[:, dt], xT[:, dt, :])

            # aggregate per-core stats, AllReduce (mean, E[x^2]) across cores
            aggo = consts.tile([128, DT, 2], f32)
            aggc = consts.tile([128, DT, 2], f32)
            for dt in range(DT):
                nc.vector.bn_aggr(aggo[:, dt, :], stato[:, dt])
                nc.vector.bn_aggr(aggc[:, dt, :], statc[:, dt])

            cc_sb = consts.tile([128, DT, 2, 2], f32)
            for dt in range(DT):
                for g, agg in enumerate((aggo, aggc)):
                    m = agg[:, dt, 0:1]
                    v = agg[:, dt, 1:2]
                    nc.vector.tensor_copy(cc_sb[:, dt, g, 0:1], m)
                    nc.vector.scalar_tensor_tensor(
                        cc_sb[:, dt, g, 1:2], in0=m, scalar=m, in1=v,
                        op0=Alu.mult, op1=Alu.add)

            cc_in = dram.tile([128, DT * 4], f32)
            cc_out = dram.tile([128, DT * 4], f32, addr_space="Shared")
            nc.gpsimd.dma_start(cc_in[:, :], cc_sb.rearrange("p a b c -> p (a b c)"))
            nc.gpsimd.collective_compute(
                "AllReduce", Alu.add,
                replica_groups=[list(range(NCORES))],
                ins=[cc_in.opt()], outs=[cc_out.opt()])
            ccr = consts.tile([128, DT, 2, 2], f32)
            nc.gpsimd.dma_start(ccr.rearrange("p a b c -> p (a b c)"), cc_out[:, :])

            gm = consts.tile([128, DT, 2], f32)     # mean   per (dt, grp)
            ge = consts.tile([128, DT, 2], f32)     # E[x^2] per (dt, grp)
            nc.vector.tensor_scalar_mul(gm.rearrange("p a b -> p (a b)"),
                                        ccr[:, :, :, 0].rearrange("p a b -> p (a b)"),
                                        1.0 / NCORES)
            nc.vector.tensor_scalar_mul(ge.rearrange("p a b -> p (a b)"),
                                        ccr[:, :, :, 1].rearrange("p a b -> p (a b)"),
                                        1.0 / NCORES)
            var4 = consts.tile([128, DT, 2], f32)
            mm4 = consts.tile([128, DT, 2], f32)
            nc.vector.tensor_mul(mm4[:, :, :], gm[:, :, :], gm[:, :, :])
            nc.vector.tensor_sub(var4[:, :, :], ge[:, :, :], mm4[:, :, :])
            eps_t = consts.tile([128, 1], f32)
            nc.vector.memset(eps_t[:, :], float(EPS))
            ln4 = consts.tile([128, DT, 2], f32)
            nc.scalar.activation(ln4.rearrange("p a b -> p (a b)"),
                                 var4.rearrange("p a b -> p (a b)"),
                                 Act.Ln, bias=eps_t[:, :])
            s4 = consts.tile([128, DT, 2], f32)     # rsqrt(var+eps)
            nc.scalar.activation(s4.rearrange("p a b -> p (a b)"),
                                 ln4.rearrange("p a b -> p (a b)"),
                                 Act.Exp, scale=-0.5)
            nms4 = consts.tile([128, DT, 2], f32)   # -mean * s
            nc.vector.scalar_tensor_tensor(
                nms4.rearrange("p a b -> p (a b)"),
                in0=gm.rearrange("p a b -> p (a b)"), scalar=-1.0,
                in1=s4.rearrange("p a b -> p (a b)"),
                op0=Alu.mult, op1=Alu.mult)

            # ---------------- fold BN into weights ----------------
            Wcb = consts.tile([128, DT, Hid], bf16)
            Wsb = consts.tile([128, DT, Hid], bf16)
            for dt in range(DT):
                nc.vector.tensor_scalar_mul(Wcb[:, dt, :], Wc32[:, dt, :],
                                            s4[:, dt, 0:1])
                nc.vector.tensor_scalar_mul(Wsb[:, dt, :], Ws32[:, dt, :],
                                            s4[:, dt, 1:2])
            if DEBUG_TAPS:
                nc.sync.dma_start(dbg_s4[:, :, :].rearrange("p a b -> p (a b)"),
                                  s4.rearrange("p a b -> p (a b)"))
                nc.sync.dma_start(dbg_nms[:, :, :].rearrange("p a b -> p (a b)"),
                                  nms4.rearrange("p a b -> p (a b)"))
            biasC = consts.tile([128, NT], f32)
            biasS = consts.tile([128, NT], f32)
            for j in range(NT):
                for bias_t, w32, base_t, g in ((biasC, Wc32, bcT, 0),
                                               (biasS, Ws32, bsT, 1)):
                    ps = psum.tile([128, 512], f32, tag="kv8", name=f"psf_{j}_{g}")
                    for dt in range(DT):
                        nc.tensor.matmul(ps[:, 0:1],
                                         lhsT=w32[:, dt, j * 128:(j + 1) * 128],
                                         rhs=nms4[:, dt, g:g + 1],
                                         start=(dt == 0), stop=(dt == DT - 1))
                    nc.scalar.activation(bias_t[:, j:j + 1], ps[:, 0:1],
                                         Act.Identity, bias=base_t[:, j:j + 1])

            # ---------------- phase B: encoders, K/V/Q, attention ----------------
            for t in range(NCH):
                xb = xb_pool.tile([128, DT, ACB], bf16, tag="xb", name=f"xb_{t}")
                for dt in range(DT):
                    nc.sync.dma_start(xb[:, dt, :], xbfT[t, dt])

                saT = sa_pool.tile([128, NT, AO * CB], bf16, tag="saT",
                                   name=f"saT_{t}")
                sa8 = sa_pool.tile([128, NT, AO * CB], f8, tag="sa8",
                                   name=f"sa8_{t}")
                sq8 = sa_pool.tile([128, NT, CB], f8, tag="sq8", name=f"sq8_{t}")
                for j in range(NT):
                    for hh in range(2):
                        lo, hi = hh * 1024, (hh + 1) * 1024
                        pcs = [p for p in pieces if lo <= p[0] < hi]
                        pe = psum.tile([128, 1024], f32, tag="enc",
                                       name=f"pse_{t}_{j}_{hh}")
                        # start=True zeroes a whole 2KB PSUM bank, so only
                        # the first piece touching a bank may set it
                        seen_banks = set()
                        for dt in range(DT):
                            for c0, cn, kind in pcs:
                                w_sb = Wcb if kind == "o" else Wsb
                                bank = (c0 - lo) // 512
                                st = dt == 0 and bank not in seen_banks
                                if dt == 0:
                                    seen_banks.add(bank)
                                nc.tensor.matmul(
                                    pe[:, c0 - lo:c0 - lo + cn],
                                    lhsT=w_sb[:, dt, j * 128:(j + 1) * 128],
                                    rhs=xb[:, dt, c0:c0 + cn],
                                    start=st, stop=(dt == DT - 1),
                                    skip_group_check=True)
                        for c0, cn, kind in pcs:
                            if kind == "o":
                                nc.scalar.activation(
                                    saT[:, j, compact(c0):compact(c0) + cn],
                                    pe[:, c0 - lo:c0 - lo + cn],
                                    Act.Prelu, bias=biasC[:, j:j + 1], alpha=ALPHA)
                            else:
                                nc.scalar.activation(
                                    sq8[:, j, :], pe[:, c0 - lo:c0 - lo + cn],
                                    Act.Prelu, bias=biasS[:, j:j + 1], alpha=ALPHA)
                    nc.vector.tensor_copy(sa8[:, j, :], saT[:, j, :])

                # Q projection (fp8 DoubleRow), drained to bf16
                pq = psum.tile([128, H * HD], f32, tag="kv8", name=f"pq_{t}")
                for g in range(NT // 2):
                    nc.tensor.matmul(pq[:, :],
                                     lhsT=sq8[:, 2 * g:2 * g + 2, :],
                                     rhs=Wq8[:, 2 * g:2 * g + 2, :],
                                     start=(g == 0), stop=(g == NT // 2 - 1),
                                     perf_mode=DR)
                q_all = at_pool.tile([128, H * HD], bf16, tag="q", name=f"q_{t}")
                nc.scalar.copy(q_all[:, :], pq[:, :])

                # K projection (fp8 DoubleRow); PSUM drain fused into the
                # q*k multiply: DVE reads K's PSUM directly per agent
                prod1 = at_pool.tile([128, H, AO, HD], bf16, tag="prod",
                                     name=f"prod1_{t}")
                q_v = q_all.rearrange("p (h d) -> p h d", h=H).unsqueeze(2)
                for ae in range(AO):
                    pk = psum.tile([128, H * HD], f32, tag="kv8",
                                   name=f"pk_{t}_{ae}")
                    for g in range(NT // 2):
                        nc.tensor.matmul(pk[:, :],
                                         lhsT=sa8[:, 2 * g:2 * g + 2,
                                                  ae * CB:(ae + 1) * CB],
                                         rhs=Wk8[:, 2 * g:2 * g + 2, :],
                                         start=(g == 0), stop=(g == NT // 2 - 1),
                                         perf_mode=DR)
                    nc.vector.tensor_mul(
                        prod1[:, :, ae:ae + 1, :],
                        pk.rearrange("p (h x d) -> p h x d", h=H, x=1),
                        q_v)

                # V projection (bf16) + prelu drain on ScalarE
                v_all = at_pool.tile([128, AO, H * HD], bf16, tag="v",
                                     bufs=1, name=f"v_{t}")
                for ae in range(AO):
                    pv = psum.tile([128, H * HD], f32, tag="vps",
                                   name=f"pv_{t}_{ae}")
                    for kt in range(NT):
                        nc.tensor.matmul(pv[:, :],
                                         lhsT=saT[:, kt, ae * CB:(ae + 1) * CB],
                                         rhs=Wv2[:, kt, :],
                                         start=(kt == 0), stop=(kt == NT - 1))
                    nc.scalar.activation(v_all[:, ae, :], pv[:, :],
                                         Act.Prelu, alpha=ALPHA)

                # scores: in-place bf16 fold tree over d (packed 4x DVE adds)
                w = HD // 2
                while w >= 1:
                    nc.vector.tensor_add(prod1[:, :, :, 0:w],
                                         prod1[:, :, :, 0:w],
                                         prod1[:, :, :, w:2 * w])
                    w //= 2

                if DEBUG_TAPS and t == 0:
                    with nc.allow_non_contiguous_dma("debug taps"):
                        nc.gpsimd.dma_start(
                            dbg_sa[:, :, :].rearrange("p j c -> p (j c)"),
                            saT.rearrange("p j c -> p (j c)"))
                        nc.gpsimd.dma_start(dbg_q[:, :], q_all[:, :])
                        nc.gpsimd.dma_start(
                            dbg_sc[:, :, :].unsqueeze(3), prod1[:, :, :, 0:1])

                # softmax (no max-subtraction: |scores/8| << 1)
                e15 = at_pool.tile([128, H, AO], bf16, tag="e15", name=f"e15_{t}")
                nc.scalar.activation(e15.unsqueeze(3), prod1[:, :, :, 0:1],
                                     Act.Exp, scale=1.0 / float(np.sqrt(HD)))
                sums = at_pool.tile([128, H], f32, tag="sums", name=f"sums_{t}")
                nc.vector.tensor_reduce(sums[:, :], e15[:, :, :],
                                        axis=mybir.AxisListType.X, op=Alu.add)
                rinv = at_pool.tile([128, H], f32, tag="rinv", name=f"rinv_{t}")
                nc.vector.reciprocal(rinv[:, :], sums[:, :])

                # att = (sum_a e*v) * rinv: e*v in one 2x op, fold tree over a
                prod2 = at_pool.tile([128, H, AO, HD], bf16, tag="prod",
                                     name=f"prod2_{t}")
                v_v = v_all.rearrange("p a (h d) -> p h a d", h=H)
                e_b = e15.unsqueeze(3).broadcast_to([128, H, AO, HD])
                nc.vector.tensor_mul(prod2[:, :, :, :], v_v, e_b)
                nc.vector.tensor_add(prod2[:, :, 0:7, :], prod2[:, :, 0:7, :],
                                     prod2[:, :, 8:15, :])
                w = 4
                while w >= 1:
                    nc.vector.tensor_add(prod2[:, :, 0:w, :],
                                         prod2[:, :, 0:w, :],
                                         prod2[:, :, w:2 * w, :])
                    w //= 2
                out_t = at_pool.tile([128, H * HD], f32, tag="out",
                                     bufs=1, name=f"out_{t}")
                r_b = rinv.unsqueeze(2).unsqueeze(3) \
                          .broadcast_to([128, H, 1, HD])
                nc.vector.tensor_mul(
                    out_t.rearrange("p (h d) -> p h d", h=H).unsqueeze(2),
                    prod2[:, :, 0:1, :], r_b)
                nc.scalar.dma_start(out_d[t * CB:(t + 1) * CB, :], out_t[:, :])

    nc.compile()
    _CACHE[ai] = nc
    return nc


def _run(inputs, trace=False, **kwargs):
    from concourse.bass_utils import run_bass_kernel_spmd

    state = np.ascontiguousarray(np.asarray(inputs["state"], dtype=np.float32))
    ai = int(np.asarray(inputs["agent_index"]))
    arrs = {}
    for name in ("Ws", "bs", "Wc", "bc", "Wk", "Wq", "Wv"):
        arrs[name] = np.ascontiguousarray(np.asarray(inputs[name], dtype=np.float32))
    arrs["ident"] = np.eye(128, dtype=np.float32)

    nc = _build(ai)
    in_maps = []
    for c in range(NCORES):
        m = {"state": np.ascontiguousarray(state[c * BL:(c + 1) * BL])}
        m.update(arrs)
        in_maps.append(m)
    res = run_bass_kernel_spmd(nc, in_maps, core_ids=list(range(NCORES)),
                               trace=trace, **kwargs)
    out = np.concatenate([r["out"] for r in res.results], axis=0).astype(np.float32)
    return out, res


def kernel(**inputs) -> np.ndarray:
    out, _ = _run(inputs, trace=False)
    return out
